# revision 1
# baseline (speedup 1.0000x reference)
"""Block-sparse (banded) attention kernel for Trainium2, 8 NeuronCores.

Sharding: data-parallel over batch (2) x tensor-parallel over heads
(16 heads -> 4 per core).  Each core computes its 4 heads' Q/K/V
projections, banded block attention (|r-c| <= 15 blocks, per-block
softmax), and a partial output projection; the host sums the 4 partial
outputs per batch element.

Self-contained: hardcodes all shapes; only needs the concourse tree that
the environment already puts on sys.path.
"""

import sys

for _p in ("/opt/trn_rl_repo",):
    if _p not in sys.path:
        sys.path.insert(0, _p)

from contextlib import ExitStack

import numpy as np

import concourse.bacc as bacc
import concourse.tile as tile
from concourse import bass_utils, mybir

F32 = mybir.dt.float32
F32R = mybir.dt.float32r
BF16 = mybir.dt.bfloat16
EXP = mybir.ActivationFunctionType.Exp

B, S, E = 2, 2048, 1024
H, HD, BLK = 16, 64, 64
NB = S // BLK  # 32 blocks
NCORES = 8
HPC = 4  # heads per core
F = HPC * HD  # 256 local features
BAND = 15
SCALE = HD ** -0.5

# per r8-slab (8 query blocks, q=512) column-block ranges, even-extended
T_SLABS = 4
QS = 512  # q extent per slab
LO = []
NP_T = []
for _t in range(T_SLABS):
    lo = max(0, 8 * _t - BAND)
    hi = min(NB - 1, 8 * _t + 7 + BAND)
    if (hi - lo + 1) % 2 == 1:
        if lo > 0:
            lo -= 1
        else:
            hi += 1
    LO.append(lo)
    NP_T.append((hi - lo + 1) // 2)
MAXP = max(NP_T)  # 16 pairs


def build_nc(debug=False):
    nc = bacc.Bacc("TRN2", target_bir_lowering=False, debug=False)

    xq_d = nc.dram_tensor("xqT", [E, S], F32R, kind="ExternalInput")
    xk_d = nc.dram_tensor("xkT", [E, S], F32R, kind="ExternalInput")
    xv_d = nc.dram_tensor("xvT", [E, S], F32R, kind="ExternalInput")
    wq_d = nc.dram_tensor("wqT", [E, F], F32R, kind="ExternalInput")
    wk_d = nc.dram_tensor("wkT", [E, F], F32R, kind="ExternalInput")
    wv_d = nc.dram_tensor("wvT", [E, F], F32R, kind="ExternalInput")
    wo_d = nc.dram_tensor("woT", [F, E], F32R, kind="ExternalInput")
    sel_d = nc.dram_tensor("selc", [128, MAXP * 32], F32R, kind="ExternalInput")
    bds_d = nc.dram_tensor("bdsel", [32, MAXP * 128], F32R, kind="ExternalInput")
    vm_d = nc.dram_tensor("vmask", [32, T_SLABS * QS], F32R, kind="ExternalInput")
    out_d = nc.dram_tensor("out", [S, E], F32, kind="ExternalOutput")
    if debug:
        qT_d = nc.dram_tensor("qT_dbg", [128, 2 * S], F32, kind="ExternalOutput")
        kT_d = nc.dram_tensor("kT_dbg", [128, 2 * S], F32, kind="ExternalOutput")
        vv_d = nc.dram_tensor("vv_dbg", [128, 16 * F], F32, kind="ExternalOutput")
        at_d = nc.dram_tensor("at_dbg", [F, S], F32, kind="ExternalOutput")

    with tile.TileContext(nc) as tc, ExitStack() as ctx, nc.allow_low_precision(
        reason="float32r pipeline; fp32 data format throughout"
    ):
        pers = ctx.enter_context(tc.tile_pool(name="pers", bufs=1))
        qT = pers.tile([128, 2 * S], F32R, tag="qT")
        kT = pers.tile([128, 2 * S], F32R, tag="kT")
        vv = pers.tile([128, 16 * F], F32R, tag="vv")
        wq = pers.tile([128, 8 * F], F32R, tag="wq")
        wk = pers.tile([128, 8 * F], F32R, tag="wk")
        wv = pers.tile([128, 8 * F], F32R, tag="wv")
        wo = pers.tile([64, 4 * E], F32R, tag="wo")
        selb = pers.tile([128, MAXP * 32], BF16, tag="selb")
        bds = pers.tile([32, MAXP * 128], F32R, tag="bds")
        vm = pers.tile([32, T_SLABS * QS], F32R, tag="vm")

        # k-projection weights first: phase 1 is on the critical path
        nc.sync.dma_start(
            wk[:].rearrange("p (c f) -> p c f", c=8),
            wk_d.ap().rearrange("(c p) f -> p c f", p=128),
        )
        # remaining weights/constants arrive via gpsimd (SWDGE) so they don't
        # queue ahead of the phase-1/2 x-tile loads on the sync ring
        nc.gpsimd.dma_start(
            wv[:].rearrange("p (c f) -> p c f", c=8),
            wv_d.ap().rearrange("(c p) f -> p c f", p=128),
        )
        nc.gpsimd.dma_start(
            wq[:].rearrange("p (c f) -> p c f", c=8),
            wq_d.ap().rearrange("(c p) f -> p c f", p=128),
        )
        nc.gpsimd.dma_start(
            wo[:].rearrange("p (c e) -> p c e", c=4),
            wo_d.ap().rearrange("(c p) e -> p c e", p=64),
        )
        nc.gpsimd.dma_start(selb[:], sel_d.ap())  # SWDGE casts f32 -> bf16
        nc.gpsimd.dma_start(bds[:], bds_d.ap())
        nc.gpsimd.dma_start(vm[:], vm_d.ap())

        # ---- phase 1: k projection (kT layout [f, s]) ----
        with tc.tile_pool(name="xk", bufs=2) as xkp, tc.tile_pool(
            name="psK", bufs=1, space="PSUM"
        ) as pskp:
            psK = pskp.tile([128, 4096], F32)
            for e in range(8):
                xt = xkp.tile([128, S], F32R, tag="xk")
                nc.sync.dma_start(xt[:], xk_d.ap()[e * 128 : (e + 1) * 128, :])
                for fold in range(2):
                    for sc in range(4):
                        nc.tensor.matmul(
                            psK[:, (fold * 4 + sc) * 512 : (fold * 4 + sc + 1) * 512],
                            wk[:, e * F + fold * 128 : e * F + fold * 128 + 128],
                            xt[:, sc * 512 : (sc + 1) * 512],
                            start=(e == 0),
                            stop=(e == 7),
                        )
            for fold in range(2):
                for sc in range(4):
                    nc.scalar.copy(
                        kT[:, fold * S + sc * 512 : fold * S + (sc + 1) * 512],
                        psK[:, (fold * 4 + sc) * 512 : (fold * 4 + sc + 1) * 512],
                    )

        # ---- phase 2: v projection (natural layout [s, f]) ----
        with tc.tile_pool(name="xv", bufs=3) as xvp, tc.tile_pool(
            name="psV", bufs=2, space="PSUM"
        ) as psvp:
            for sc in range(4):
                # one PSUM bank per sub-chunk: accumulation groups must not
                # interleave within a bank
                pvs = [
                    psvp.tile([128, 256], F32, name=f"pv{sub}", tag=f"psV{sub}")
                    for sub in range(4)
                ]
                for e in range(8):
                    xt = xvp.tile([128, 512], F32R, tag="xv")
                    nc.sync.dma_start(
                        xt[:],
                        xv_d.ap()[e * 128 : (e + 1) * 128, sc * 512 : (sc + 1) * 512],
                    )
                    for sub in range(4):
                        nc.tensor.matmul(
                            pvs[sub][:],
                            xt[:, sub * 128 : (sub + 1) * 128],
                            wv[:, e * F : (e + 1) * F],
                            start=(e == 0),
                            stop=(e == 7),
                        )
                for sub in range(4):
                    nc.scalar.copy(
                        vv[:, sc * 1024 + sub * 256 : sc * 1024 + (sub + 1) * 256],
                        pvs[sub][:],
                    )

        # ---- phase 3: q projection + attention + output projection ----
        xqp = ctx.enter_context(tc.tile_pool(name="xq", bufs=3))
        psSp = ctx.enter_context(tc.tile_pool(name="psS", bufs=6, space="PSUM"))
        
        flexp = ctx.enter_context(tc.tile_pool(name="flex", bufs=2, space="PSUM"))
        expp = ctx.enter_context(tc.tile_pool(name="expS", bufs=2))
        ptp = ctx.enter_context(tc.tile_pool(name="pt", bufs=4))
        rcpp = ctx.enter_context(tc.tile_pool(name="rcp", bufs=2))
        attp = ctx.enter_context(tc.tile_pool(name="att", bufs=8))
        outp = ctx.enter_context(tc.tile_pool(name="outsb", bufs=2))

        def unitA(h, t):
            npt = NP_T[t]
            lo = LO[t]
            fold = h // 2
            bp = 64 * (h % 2)  # partition base of this head's qT/kT rows
            expS = expp.tile([128, MAXP * QS], BF16, tag="expS")
            accs = psSp.tile([128, 512], F32, name="accs", tag="psS")
            for j in range(npt):
                c0 = lo + 2 * j
                ps = psSp.tile([128, 512], F32, name="ps", tag="psS")
                nc.tensor.matmul(
                    ps[:],
                    kT[bp : bp + 64, fold * S + c0 * 64 : fold * S + c0 * 64 + 128],
                    qT[bp : bp + 64, fold * S + t * QS : fold * S + (t + 1) * QS],
                    start=True,
                    stop=True,
                )
                nc.scalar.activation(
                    expS[:, j * QS : (j + 1) * QS], ps[:], EXP
                )
                nc.tensor.matmul(
                    accs[0:32, :],
                    selb[:, j * 32 : (j + 1) * 32],
                    expS[:, j * QS : (j + 1) * QS],
                    start=(j == 0),
                    stop=(j == npt - 1),
                )
            return expS, accs

        def unitB(h, t, expS, accs, attn_t):
            npt = NP_T[t]
            lo = LO[t]
            acco = psSp.tile([128, 512], F32, name="acco", tag="psS")
            rc = rcpp.tile([32, 512], F32R, tag="rcp")
            rs1 = rcpp.tile([32, 512], F32, tag="rcs1")
            rs2 = rcpp.tile([32, 512], F32, tag="rcs2")
            nc.vector.reciprocal_approx_accurate(rs2[:], accs[0:32, :], rs1[:])
            nc.vector.tensor_mul(rc[:], rs2[:], vm[:, t * QS : (t + 1) * QS])
            for j in range(npt):
                bt = flexp.tile([128, 512], F32, tag="flex")
                nc.tensor.matmul(
                    bt[:],
                    bds[0 : 2 * npt, j * 128 : (j + 1) * 128],
                    rc[0 : 2 * npt, :],
                    start=True,
                    stop=True,
                )
                pt = ptp.tile([128, 512], F32R, tag="pt")
                nc.vector.tensor_mul(pt[:], expS[:, j * QS : (j + 1) * QS], bt[:])
                cp = lo // 2 + j
                nc.tensor.matmul(
                    acco[0:64, :],
                    vv[:, cp * F + h * 64 : cp * F + h * 64 + 64],
                    pt[:],
                    start=(j == 0),
                    stop=(j == npt - 1),
                )
            nc.scalar.copy(attn_t[:, :], acco[0:64, :])

        def outproj(t, atts):
            for sc2 in range(4):
                ob = outp.tile([128, 1024], F32, tag="outsb")
                for eh in range(2):
                    po = flexp.tile([128, 512], F32, tag="flex")
                    for h in range(HPC):
                        nc.tensor.matmul(
                            po[:],
                            atts[h][:, sc2 * 128 : sc2 * 128 + 128],
                            wo[:, h * E + eh * 512 : h * E + eh * 512 + 512],
                            start=(h == 0),
                            stop=(h == HPC - 1),
                        )
                    nc.scalar.copy(ob[:, eh * 512 : (eh + 1) * 512], po[:])
                row = (4 * t + sc2) * 128
                nc.gpsimd.dma_start(out_d.ap()[row : row + 128, :], ob[:])

        def qproj(sc4):
            pqs = [
                psSp.tile([128, 512], F32, name=f"pq{fold}", tag="psS")
                for fold in range(2)
            ]
            for e in range(8):
                xt = xqp.tile([128, 512], F32R, tag="xq")
                nc.sync.dma_start(
                    xt[:],
                    xq_d.ap()[e * 128 : (e + 1) * 128, sc4 * 512 : (sc4 + 1) * 512],
                )
                for fold in range(2):
                    nc.tensor.matmul(
                        pqs[fold][:],
                        wq[:, e * F + fold * 128 : e * F + fold * 128 + 128],
                        xt[:],
                        start=(e == 0),
                        stop=(e == 7),
                    )
            for fold in range(2):
                nc.scalar.copy(
                    qT[:, fold * S + sc4 * 512 : fold * S + (sc4 + 1) * 512],
                    pqs[fold][:],
                )

        units = [(t, h) for t in range(T_SLABS) for h in range(HPC)]
        pending = None
        atts_by_t = {t: [] for t in range(T_SLABS)}
        for t, h in units:
            if h == 0:
                qproj(t)
            stA = unitA(h, t)
            if pending is not None:
                pt_, ph_, pexpS, pacc, pattn = pending
                unitB(ph_, pt_, pexpS, pacc, pattn)
                atts_by_t[pt_].append(pattn)
                if debug:
                    nc.gpsimd.dma_start(
                        at_d.ap()[ph_ * 64 : ph_ * 64 + 64, pt_ * QS : (pt_ + 1) * QS],
                        pattn[:],
                    )
                if len(atts_by_t[pt_]) == HPC:
                    outproj(pt_, atts_by_t[pt_])
            attn_t = attp.tile([64, 512], F32R, tag="att")
            pending = (t, h, stA[0], stA[1], attn_t)
        pt_, ph_, pexpS, pacc, pattn = pending
        unitB(ph_, pt_, pexpS, pacc, pattn)
        atts_by_t[pt_].append(pattn)
        if debug:
            nc.gpsimd.dma_start(
                at_d.ap()[ph_ * 64 : ph_ * 64 + 64, pt_ * QS : (pt_ + 1) * QS],
                pattn[:],
            )
        outproj(pt_, atts_by_t[pt_])

        if debug:
            nc.gpsimd.dma_start(qT_d.ap(), qT[:])
            nc.gpsimd.dma_start(kT_d.ap(), kT[:])
            nc.gpsimd.dma_start(vv_d.ap(), vv[:])

    nc.compile()
    return nc


_NC_CACHE = []


def _get_nc():
    if not _NC_CACHE:
        _NC_CACHE.append(build_nc())
    return _NC_CACHE[0]


def _host_consts():
    selc = np.zeros((128, MAXP * 32), np.float32)
    for k in range(128):
        for j in range(MAXP):
            selc[k, j * 32 + 2 * j + k // 64] = 1.0
    bdsel = np.zeros((32, MAXP * 128), np.float32)
    for j in range(MAXP):
        for p in range(128):
            bdsel[2 * j + p // 64, j * 128 + p] = 1.0
    vmask = np.zeros((32, T_SLABS * QS), np.float32)
    for t in range(T_SLABS):
        for m in range(2 * NP_T[t]):
            c = LO[t] + m
            for qb in range(QS // BLK):
                r = 8 * t + qb
                if abs(r - c) <= BAND:
                    vmask[m, t * QS + qb * 64 : t * QS + (qb + 1) * 64] = 1.0
    return selc, bdsel, vmask


def kernel(query, key, value, Wq, Wk, Wv, Wo):
    query = np.asarray(query, np.float32)
    key = np.asarray(key, np.float32)
    value = np.asarray(value, np.float32)
    Wq = np.asarray(Wq, np.float32)
    Wk = np.asarray(Wk, np.float32)
    Wv = np.asarray(Wv, np.float32)
    Wo = np.asarray(Wo, np.float32)

    nc = _get_nc()
    selc, bdsel, vmask = _host_consts()

    in_maps = []
    for c in range(NCORES):
        b, g = divmod(c, HPC)
        fs = slice(F * g, F * (g + 1))
        in_maps.append(
            {
                "xqT": np.ascontiguousarray(query[b].T),
                "xkT": np.ascontiguousarray(key[b].T),
                "xvT": np.ascontiguousarray(value[b].T),
                "wqT": np.ascontiguousarray((Wq[fs, :] * SCALE).T),
                "wkT": np.ascontiguousarray(Wk[fs, :].T),
                "wvT": np.ascontiguousarray(Wv[fs, :].T),
                "woT": np.ascontiguousarray(Wo[:, fs].T),
                "selc": selc,
                "bdsel": bdsel,
                "vmask": vmask,
            }
        )

    res = bass_utils.run_bass_kernel_spmd(nc, in_maps, core_ids=list(range(NCORES)))
    out = np.zeros((B, S, E), np.float32)
    for c in range(NCORES):
        b = c // HPC
        out[b] += res.results[c]["out"]
    return out

